# revision 22
# baseline (speedup 1.0000x reference)
"""Ragged cross-attention pooling kernel for Trainium2 (8 NeuronCores, SPMD).

Math (per pair, direction "A attends over B"):
    qa = (A @ Wq + bq) * scale          [la, INNER]
    kb =  B @ Wk                        [lb, INNER]   (bk dropped: softmax
                                                       is shift-invariant per query)
    s  = qa @ kb^T                      [la, lb]      (pad k-cols are exactly 0)
    p  = exp(s)                                       (pad cols: exp(0) = 1.0)
    den[q] = sum_k p[q, k] - n_pad                    (exact pad correction)
    g[q] = valid(q) / (la * den[q])
    w[k] = sum_q g[q] p[q, k]           <- collapses the mean over queries
    emb  = (w^T B) @ Wv + bv            <- collapses attn@V and the V projection

Distribution: 64 pairs -> 8 slots x 8 cores (one shared SPMD program, shapes
fixed per slot to the max over cores; pairs bin-packed by length so padding is
small).

Perf notes vs the first version:
  - A/B uploaded BOTH pre-transposed (DIM-major, fp8e4m3, DIM zero-padded to
    768) for the Q/K path AND natural-layout bf16 for the value path; no
    on-device transposes at all.
  - Projections and QK^T run as fp8 DoubleRow matmuls (2 contraction rows per
    partition, 0.5 cyc/row).  INNER=256 = 2x128 maps exactly onto the
    DoubleRow pair dim for the scores.
  - Value path (w^T B, Wv^T u) in bf16 (1 cyc/row, no small-N penalty).
  - exp() is one activation per q-tile over a [128, plk] PSUM span with a
    single accumulator read for den.
"""

import os
import sys

sys.path.insert(0, "/opt/trn_rl_repo")

import numpy as np

B, LA, LB, DIM, INNER, OUTER = 64, 1024, 1024, 640, 256, 1024
NCORES, NSLOTS, P = 8, 8, 128
SCALE = 1.0 / np.sqrt(INNER)
DT = DIM // P       # 5 d-chunks of 128
DJ = 3              # DoubleRow d-pair chunks (768 = 3 * 256)
DPAD = DJ * 2 * P   # 768

LAST_EXEC_TIME_NS = None


def _chunks(total, step=512):
    out, off = [], 0
    while off < total:
        c = min(step, total - off)
        out.append((off, c))
        off += c
    return out


def _plan(la_all, lb_all):
    """Assign pairs to (slot, core); returns swap flags, groups, slot tile shapes."""
    la = np.asarray(la_all, np.int64)
    lb = np.asarray(lb_all, np.int64)
    swap = lb > la
    qa = np.where(swap, lb, la)  # kernel A-side length (>= B-side)
    qb = np.where(swap, la, lb)
    at = -(-qa // P)
    bt = -(-qb // P)
    order = np.argsort(-(at * 1024 + bt), kind="stable")
    groups = [list(order[s * NCORES:(s + 1) * NCORES]) for s in range(NSLOTS)]
    C1, C2 = 2000.0, 200.0

    def gcost(g):
        ma = max(at[i] for i in g)
        mb = max(bt[i] for i in g)
        return C1 * (ma + mb) + C2 * ma * mb

    rng = np.random.default_rng(0)
    cost = [gcost(g) for g in groups]
    s1s = rng.integers(0, NSLOTS, 30000)
    s2s = rng.integers(0, NSLOTS, 30000)
    i1s = rng.integers(0, NCORES, 30000)
    i2s = rng.integers(0, NCORES, 30000)
    for s1, s2, i1, i2 in zip(s1s, s2s, i1s, i2s):
        if s1 == s2:
            continue
        g1 = groups[s1][:]
        g2 = groups[s2][:]
        g1[i1], g2[i2] = groups[s2][i2], groups[s1][i1]
        n1, n2 = gcost(g1), gcost(g2)
        if n1 + n2 < cost[s1] + cost[s2] - 1e-9:
            groups[s1], groups[s2] = g1, g2
            cost[s1], cost[s2] = n1, n2
    slot_at = [int(max(at[i] for i in g)) for g in groups]
    slot_bt = [int(max(bt[i] for i in g)) for g in groups]
    return swap, qa, qb, groups, slot_at, slot_bt


def _build_program(slot_at, slot_bt):
    import concourse.bass as bass  # noqa: F401
    import concourse.mybir as mybir
    import concourse.tile as tile
    from concourse import bacc

    F32 = mybir.dt.float32
    F32R = mybir.dt.float32r
    BF16 = mybir.dt.bfloat16
    FP8 = mybir.dt.float8e4
    Exp = mybir.ActivationFunctionType.Exp
    Ident = mybir.ActivationFunctionType.Identity
    DR = mybir.MatmulPerfMode.DoubleRow
    Alu = mybir.AluOpType

    tot_at = sum(slot_at)
    tot_bt = sum(slot_bt)
    cum_at = np.concatenate([[0], np.cumsum(slot_at)]).astype(int)
    cum_bt = np.concatenate([[0], np.cumsum(slot_bt)]).astype(int)

    nc = bacc.Bacc("TRN2", target_bir_lowering=False, debug=False,
                   num_devices=NCORES)

    # transposed fp8 inputs: [p, h, tok] = X[tok, h*128 + p]; the 6th
    # half-pair (zeros + the bias ones-row) is synthesized on device
    at8_d = nc.dram_tensor("at8", [P, 5, tot_at * P], FP8,
                           kind="ExternalInput")
    bt8_d = nc.dram_tensor("bt8", [P, 5, tot_bt * P], FP8,
                           kind="ExternalInput")
    # natural bf16 inputs: [p, T, d] = X[T*128 + p, d]
    an_d = nc.dram_tensor("an16", [P, tot_at, DIM], BF16, kind="ExternalInput")
    bn_d = nc.dram_tensor("bn16", [P, tot_bt, DIM], BF16, kind="ExternalInput")
    gs_a_d = nc.dram_tensor("gs_a", [P, tot_at], F32, kind="ExternalInput")
    gs_b_d = nc.dram_tensor("gs_b", [P, tot_bt], F32, kind="ExternalInput")
    npa_d = nc.dram_tensor("npa", [P, NSLOTS], F32, kind="ExternalInput")
    npb_d = nc.dram_tensor("npb", [P, NSLOTS], F32, kind="ExternalInput")
    wq_d = nc.dram_tensor("wq8", [P, DJ, 2, INNER], FP8, kind="ExternalInput")
    wk_d = nc.dram_tensor("wk8", [P, DJ, 2, INNER], FP8, kind="ExternalInput")
    wv_d = nc.dram_tensor("wv16", [P, DT, OUTER], BF16, kind="ExternalInput")
    bv_d = nc.dram_tensor("bv", [P, OUTER // P], F32, kind="ExternalInput")
    idr_d = nc.dram_tensor("idr", [P, P], F32R, kind="ExternalInput")
    idb_d = nc.dram_tensor("idb", [P, P], BF16, kind="ExternalInput")
    emb_d = nc.dram_tensor("emb", [P, OUTER // P, 2 * NSLOTS], F32,
                           kind="ExternalOutput")

    with tile.TileContext(nc) as tc:
        with (
            tc.tile_pool(name="const", bufs=1) as cpool,
            tc.tile_pool(name="ain", bufs=3) as apool,
            tc.tile_pool(name="proj", bufs=2) as ppool,
            tc.tile_pool(name="pexp", bufs=12) as epool,
            tc.tile_pool(name="small", bufs=7) as spool,
            tc.tile_pool(name="late", bufs=2) as lpool,
            tc.tile_pool(name="psA", bufs=3, space="PSUM") as psA,
            tc.tile_pool(name="psW", bufs=1, space="PSUM") as psW,
        ):
            # ---- constants (DMA-ordered: slot-0 critical path first) ----
            wq_sb = cpool.tile([P, DJ, 2, INNER], FP8, tag="wq")
            wk_sb = cpool.tile([P, DJ, 2, INNER], FP8, tag="wk")
            bv_sb = cpool.tile([P, OUTER // P], F32, tag="bv")
            idr_sb = cpool.tile([P, P], F32R, tag="idr")
            npa_sb = cpool.tile([P, NSLOTS], F32, tag="npa")
            npb_sb = cpool.tile([P, NSLOTS], F32, tag="npb")
            gs_a_sb = cpool.tile([P, tot_at], F32, tag="gsa")
            gs_b_sb = cpool.tile([P, tot_bt], F32, tag="gsb")
            wv_sb = cpool.tile([P, DT, OUTER], BF16, tag="wv")
            urows_sb = cpool.tile([2 * NSLOTS, DIM], F32R, tag="urows")
            for sb, d in ((wq_sb, wq_d), (wk_sb, wk_d)):
                nc.sync.dma_start(sb[:], d[:])

            inbufs = {}

            def load_slot(s, qk_only=False, nat_only=False):
                at_s, bt_s = int(slot_at[s]), int(slot_bt[s])
                if not nat_only:
                    a8 = apool.tile([P, 6, at_s * P], FP8, tag="a8")
                    b8 = apool.tile([P, 6, bt_s * P], FP8, tag="b8")
                    nc.sync.dma_start(
                        a8[:, 0:5, :],
                        at8_d[:, :, cum_at[s] * P:(cum_at[s] + at_s) * P])
                    nc.sync.dma_start(
                        b8[:, 0:5, :],
                        bt8_d[:, :, cum_bt[s] * P:(cum_bt[s] + bt_s) * P])
                    for t8 in (a8, b8):
                        nc.gpsimd.memset(t8[:, 5, :], 0.0)
                        nc.gpsimd.memset(t8[0:1, 5, :], 1.0)
                    inbufs[s] = (a8, b8, None, None)
                if not qk_only:
                    an = apool.tile([P, at_s, DIM], BF16, tag="an")
                    bn = apool.tile([P, bt_s, DIM], BF16, tag="bn")
                    nc.sync.dma_start(an[:],
                                      an_d[:, cum_at[s]:cum_at[s] + at_s, :])
                    nc.sync.dma_start(bn[:],
                                      bn_d[:, cum_bt[s]:cum_bt[s] + bt_s, :])
                    a8, b8, _, _ = inbufs[s]
                    inbufs[s] = (a8, b8, an, bn)

            projbufs = {}

            def proj_gen(s):
                """fp8 DoubleRow projections: qT/kT [p, m, tok]."""
                at_s, bt_s = int(slot_at[s]), int(slot_bt[s])
                pla, plb = at_s * P, bt_s * P
                a8, b8, an, bn = inbufs.pop(s)
                qaT = ppool.tile([P, 2, pla], FP8, tag="qaT")
                kaT = ppool.tile([P, 2, pla], FP8, tag="kaT")
                qbT = ppool.tile([P, 2, plb], FP8, tag="qbT")
                kbT = ppool.tile([P, 2, plb], FP8, tag="kbT")
                projbufs[s] = (qaT, kaT, qbT, kbT, an, bn)
                for src, pl, dst, w_sb in (
                        (a8, pla, qaT, wq_sb),
                        (a8, pla, kaT, wk_sb),
                        (b8, plb, qbT, wq_sb),
                        (b8, plb, kbT, wk_sb)):
                    for m in range(2):
                        pp = psA.tile([P, 1024], F32, tag="mm")
                        for j in range(DJ):
                            for co, cl in _chunks(pl):
                                nc.tensor.matmul(
                                    pp[:, co:co + cl],
                                    w_sb[:, j, :, m * P:(m + 1) * P],
                                    src[:, 2 * j:2 * j + 2, co:co + cl],
                                    start=(j == 0), stop=(j == DJ - 1),
                                    perf_mode=DR)
                        # plain fp8 cast: the q bias rides the ones-row of
                        # A/Wq, the softmax scale rides the exp activation,
                        # and k needs no bias (softmax shift-invariance)
                        nc.vector.tensor_copy(dst[:, m, :], pp[:, :pl])
                        yield

            def tail_gen(s, dr, wr, plk, nk, knat):
                """Deferred per-direction epilogue: transpose w, compute u.

                First step (run eagerly at direction end): wrow copy, freeing
                the wr psum slot.  Later steps are drained one per q-tile
                of the following direction so the PE never idles on the
                wrow/wcol dependency chain.
                """
                wrow = lpool.tile([1, 1024], F32R, tag="wrow")
                if dr == 0:
                    nc.scalar.copy(wrow[0:1, :plk], wr[0:1, :plk])
                else:
                    nc.vector.tensor_copy(wrow[0:1, :plk], wr[0:1, :plk])
                yield
                wt = psA.tile([P, 1024], F32, tag="mm")
                for kt in range(nk):
                    nc.tensor.matmul(
                        wt[:, 2 * kt:2 * kt + 2],
                        wrow[0:1, kt * P:(kt + 1) * P],
                        idr_sb[0:1, 0:2], start=True, stop=True)
                wcol = spool.tile([P, 8], BF16, tag="wcol")
                nc.vector.tensor_copy(
                    wcol[:, :nk],
                    wt[:, :2 * nk].rearrange(
                        "p (k two) -> p k two", two=2)[:, :, 0])
                yield
                # u row = w^T @ Knat   (bf16); ur reuses the wr psum slot
                ur = psW.tile([1, 1024], F32, tag="wr")
                for co, cl in _chunks(DIM):
                    for kt in range(nk):
                        nc.tensor.matmul(
                            ur[0:1, co:co + cl],
                            wcol[:, kt:kt + 1],
                            knat[:, kt, co:co + cl],
                            start=(kt == 0), stop=(kt == nk - 1))
                ursb = lpool.tile([1, DIM], F32R, tag="ursb")
                if dr == 0:
                    nc.vector.tensor_copy(ursb[:], ur[0:1, :DIM])
                else:
                    nc.scalar.copy(ursb[:], ur[0:1, :DIM])
                nc.sync.dma_start(urows_sb[2 * s + dr:2 * s + dr + 1, :],
                                  ursb[:])

            # unified deferred-work queue: wacc pairs and direction tails are
            # issued ~2 q-tiles late (crossing direction/slot boundaries) so
            # the scalar->vector dependency chain never stalls the in-order
            # PE queue
            work = []

            def drain_work(slack):
                # Pops exhausted items freely; steps the head generator, but
                # keeps stepping whenever the backlog exceeds the hard bound
                # so ring-buffer reuse distances stay within the pool sizes.
                while len(work) > slack:
                    gen = work[0]
                    if next(gen, StopIteration) is StopIteration:
                        work.pop(0)
                        continue
                    if slack and len(work) <= 4:
                        break

            def attn_gen(s):
                at_s, bt_s = int(slot_at[s]), int(slot_bt[s])
                qaT, kaT, qbT, kbT, an, bn = projbufs.pop(s)
                for dr in range(2):
                    if dr == 0:  # A queries over B keys
                        QT, KT, nq, nk = qaT, kbT, at_s, bt_s
                        g_sb, g_off = gs_a_sb, cum_at[s]
                        np_sb = npb_sb
                        knat = bn
                    else:
                        QT, KT, nq, nk = qbT, kaT, bt_s, at_s
                        g_sb, g_off = gs_b_sb, cum_bt[s]
                        np_sb = npa_sb
                        knat = an
                    plk = nk * P
                    kch = _chunks(plk)
                    wr = psW.tile([1, 1024], F32, tag="wr")
                    den2 = None

                    def wacc_gen(q0, qn, gcol2, ptiles, wr=wr, kch=kch,
                                 nq=nq):
                        for qp in range(q0, qn + 1):
                            pt = ptiles[qp]
                            for co, cl in kch:
                                nc.tensor.matmul(
                                    wr[0:1, co:co + cl],
                                    gcol2[:, qp - q0:qp - q0 + 1],
                                    pt[:, co:co + cl],
                                    start=(qp == 0), stop=(qp == nq - 1))
                        return
                        yield

                    p_tiles = {}
                    for qt in range(nq):
                        sc = psA.tile([P, 1024], F32, tag="mm")
                        for co, cl in kch:
                            nc.tensor.matmul(
                                sc[:, co:co + cl],
                                QT[:, :, qt * P:(qt + 1) * P],
                                KT[:, :, co:co + cl],
                                start=True, stop=True, perf_mode=DR)
                        if qt % 2 == 0:
                            den2 = spool.tile([P, 2], F32, tag="den")
                        p_sb = epool.tile([P, 1024], BF16, tag="p")
                        p_tiles[qt] = p_sb
                        # p = exp(s / sqrt(INNER)); the softmax scale rides
                        # the activation, the q bias rides the ones-row of A
                        nc.scalar.activation(
                            p_sb[:, :plk], sc[:, :plk], Exp, scale=SCALE,
                            accum_out=den2[:, qt % 2:qt % 2 + 1])
                        if qt % 2 == 1 or qt == nq - 1:
                            q0 = qt - (qt % 2)
                            npair = qt - q0 + 1
                            dpair = den2[:, :npair]
                            # den -= pad count (pad cols are exactly exp(0)=1)
                            nc.vector.tensor_scalar_sub(
                                dpair, dpair, np_sb[:, s:s + 1])
                            rec2 = spool.tile([P, 2], F32, tag="rec")
                            nc.vector.reciprocal(rec2[:, :npair], dpair)
                            gcol2 = spool.tile([P, 2], BF16, tag="gc")
                            nc.vector.tensor_tensor(
                                gcol2[:, :npair], rec2[:, :npair],
                                g_sb[:, g_off + q0:g_off + q0 + npair],
                                Alu.mult)
                            work.append(wacc_gen(q0, qt, gcol2, p_tiles))
                            p_tiles = {}
                        drain_work(3)
                        yield
                    work.append(tail_gen(s, dr, wr, plk, nk, knat))
                    yield

            # software pipeline: slot attention interleaved with the next
            # slot's projections; input DMA prefetched ~two slots ahead;
            # slots processed smallest-first so the pipeline warm-up bubble
            # is as short as possible
            sorder = sorted(range(NSLOTS),
                            key=lambda s: slot_at[s] + slot_bt[s])
            load_slot(sorder[0], qk_only=True)
            for sb, d in ((gs_a_sb, gs_a_d), (gs_b_sb, gs_b_d),
                          (npa_sb, npa_d), (npb_sb, npb_d),
                          (idr_sb, idr_d)):
                nc.sync.dma_start(sb[:], d[:])
            load_slot(sorder[0], nat_only=True)
            load_slot(sorder[1])
            nc.sync.dma_start(bv_sb[:], bv_d[:])
            nc.sync.dma_start(wv_sb[:], wv_d[:])
            for _ in proj_gen(sorder[0]):
                pass
            for i, s in enumerate(sorder):
                if i + 2 < NSLOTS:
                    load_slot(sorder[i + 2])
                ag = attn_gen(s)
                pg = proj_gen(sorder[i + 1]) if i + 1 < NSLOTS else None
                for _ in ag:
                    if pg is not None:
                        if next(pg, StopIteration) is StopIteration:
                            pg = None
                if pg is not None:
                    for _ in pg:
                        pass
            drain_work(0)

            # ---- final: E = Wv^T U + bv ----
            u_sb = cpool.tile([P, DT, 2 * NSLOTS], BF16, tag="usb")
            for dt in range(DT):
                ut = psA.tile([P, 1024], F32, tag="mm")
                nc.tensor.matmul(
                    ut[:, :2 * NSLOTS],
                    urows_sb[:, dt * P:(dt + 1) * P],
                    idr_sb[0:2 * NSLOTS, 0:2 * NSLOTS],
                    start=True, stop=True)
                nc.vector.tensor_copy(u_sb[:, dt, :], ut[:, :2 * NSLOTS])
            e_sb = cpool.tile([P, OUTER // P, 2 * NSLOTS], F32, tag="esb")
            for oc in range(OUTER // P):
                ep = psA.tile([P, 1024], F32, tag="mm")
                for dt in range(DT):
                    nc.tensor.matmul(
                        ep[:, :2 * NSLOTS],
                        wv_sb[:, dt, oc * P:(oc + 1) * P],
                        u_sb[:, dt, :],
                        start=(dt == 0), stop=(dt == DT - 1))
                nc.vector.tensor_scalar_add(e_sb[:, oc, :], ep[:, :2 * NSLOTS],
                                            bv_sb[:, oc, None])
            nc.sync.dma_start(emb_d[:], e_sb[:])

    nc.compile()
    return nc


def _install_profhook():
    import contextlib
    import ctypes
    import types

    import antenv

    if not hasattr(antenv, "axon_hooks"):
        mod = types.ModuleType("antenv.axon_hooks")
        mod._hook = None

        def _set(h):
            mod._hook = h

        def _get():
            return mod._hook

        mod.set_axon_ntff_profile_hook = _set
        mod.get_axon_ntff_profile_hook = _get
        sys.modules["antenv.axon_hooks"] = mod
        antenv.axon_hooks = mod
    from antenv.axon_hooks import set_axon_ntff_profile_hook
    so_path = "/opt/axon/libaxon_pjrt.so"
    if not os.path.exists(so_path):
        return False
    lib = ctypes.CDLL(so_path)
    if not hasattr(lib, "axon_start_nrt_profile"):
        return False
    lib.axon_start_nrt_profile.argtypes = [ctypes.POINTER(ctypes.c_int64),
                                           ctypes.c_size_t]
    lib.axon_start_nrt_profile.restype = ctypes.c_int64
    lib.axon_stop_nrt_profile.argtypes = [ctypes.c_char_p]
    lib.axon_stop_nrt_profile.restype = ctypes.c_int64

    @contextlib.contextmanager
    def _hook(output_dir, device_ids):
        import jax

        jax.devices()
        if device_ids:
            ids = (ctypes.c_int64 * len(device_ids))(*device_ids)
            rc = lib.axon_start_nrt_profile(ids, len(device_ids))
        else:
            rc = lib.axon_start_nrt_profile(None, 0)
        if rc != 0:
            raise RuntimeError(f"axon_start_nrt_profile rc={rc}")
        try:
            yield
        finally:
            n = lib.axon_stop_nrt_profile(str(output_dir).encode())
            print(f"profile: {n} file(s) written to {output_dir}",
                  file=sys.stderr)

    set_axon_ntff_profile_hook(_hook)
    return True


def kernel(a_pad, b_pad, len_a, len_b, Wq, bq, Wk, bk, Wv, bv):
    global LAST_EXEC_TIME_NS
    import ml_dtypes
    FP8 = ml_dtypes.float8_e4m3fn
    BF16 = ml_dtypes.bfloat16

    a_pad = np.ascontiguousarray(np.asarray(a_pad, np.float32))
    b_pad = np.ascontiguousarray(np.asarray(b_pad, np.float32))
    len_a = np.asarray(len_a, np.int32)
    len_b = np.asarray(len_b, np.int32)
    Wq = np.asarray(Wq, np.float32)
    Wk = np.asarray(Wk, np.float32)
    Wv = np.asarray(Wv, np.float32)
    bq = np.asarray(bq, np.float32)
    bv = np.asarray(bv, np.float32)

    swap, qa_len, qb_len, groups, slot_at, slot_bt = _plan(len_a, len_b)
    tot_at, tot_bt = sum(slot_at), sum(slot_bt)
    cum_at = np.concatenate([[0], np.cumsum(slot_at)]).astype(int)
    cum_bt = np.concatenate([[0], np.cumsum(slot_bt)]).astype(int)

    # ---- shared (per-core-identical) inputs ----
    def pack_w8(W, brow=None):
        # [640, INNER] -> [128, 3, 2, INNER] with d = j*256 + i*128 + p;
        # row DIM carries the bias (the data carries 1.0 there)
        Wp = np.zeros((DPAD, W.shape[1]), np.float32)
        Wp[:DIM] = W
        if brow is not None:
            Wp[DIM] = brow
        return np.ascontiguousarray(
            Wp.reshape(DJ, 2, P, W.shape[1]).transpose(2, 0, 1, 3)
        ).astype(FP8)

    wq8 = pack_w8(Wq, bq)
    wk8 = pack_w8(Wk)
    wv16 = np.ascontiguousarray(
        Wv.reshape(DT, P, OUTER).transpose(1, 0, 2)).astype(BF16)
    bv_h = bv.reshape(OUTER // P, P).T.copy()
    idr_h = np.eye(P, dtype=np.float32)
    idb_h = np.eye(P, dtype=np.float32).astype(BF16)

    # ---- per-core inputs ----
    in_maps = []
    for c in range(NCORES):
        abuf = np.zeros((tot_at * P, DIM), np.float32)
        bbuf = np.zeros((tot_bt * P, DIM), np.float32)
        gs_a = np.zeros((P, tot_at), np.float32)
        gs_b = np.zeros((P, tot_bt), np.float32)
        npa = np.zeros((P, NSLOTS), np.float32)
        npb = np.zeros((P, NSLOTS), np.float32)
        for s in range(NSLOTS):
            i = groups[s][c]
            la_i, lb_i = int(qa_len[i]), int(qb_len[i])
            A = b_pad[i] if swap[i] else a_pad[i]
            Bm = a_pad[i] if swap[i] else b_pad[i]
            abuf[cum_at[s] * P:cum_at[s] * P + la_i] = A[:la_i]
            bbuf[cum_bt[s] * P:cum_bt[s] * P + lb_i] = Bm[:lb_i]
            ga = np.zeros(slot_at[s] * P, np.float32)
            ga[:la_i] = 1.0 / la_i
            gs_a[:, cum_at[s]:cum_at[s] + slot_at[s]] = \
                ga.reshape(slot_at[s], P).T
            gb = np.zeros(slot_bt[s] * P, np.float32)
            gb[:lb_i] = 1.0 / lb_i
            gs_b[:, cum_bt[s]:cum_bt[s] + slot_bt[s]] = \
                gb.reshape(slot_bt[s], P).T
            npa[:, s] = slot_at[s] * P - la_i
            npb[:, s] = slot_bt[s] * P - lb_i
        # transposed fp8: [tok, 640] -> [128, 5, tok]
        at8 = np.ascontiguousarray(
            abuf.reshape(tot_at * P, 5, P).transpose(2, 1, 0)).astype(FP8)
        bt8 = np.ascontiguousarray(
            bbuf.reshape(tot_bt * P, 5, P).transpose(2, 1, 0)).astype(FP8)
        # natural bf16: [tok, 640] -> [128, T, 640]
        an16 = np.ascontiguousarray(
            abuf.reshape(tot_at, P, DIM).transpose(1, 0, 2)).astype(BF16)
        bn16 = np.ascontiguousarray(
            bbuf.reshape(tot_bt, P, DIM).transpose(1, 0, 2)).astype(BF16)
        in_maps.append({
            "at8": at8, "bt8": bt8, "an16": an16, "bn16": bn16,
            "gs_a": gs_a, "gs_b": gs_b, "npa": npa, "npb": npb,
            "wq8": wq8, "wk8": wk8, "wv16": wv16,
            "bv": bv_h, "idr": idr_h, "idb": idb_h,
        })

    nc = _build_program(slot_at, slot_bt)

    from concourse.bass_utils import run_bass_kernel_spmd

    trace = os.environ.get("BASS_KERNEL_TRACE", "0") == "1"
    if trace:
        _install_profhook()
    res = run_bass_kernel_spmd(nc, in_maps, list(range(NCORES)), trace=trace)
    LAST_EXEC_TIME_NS = res.exec_time_ns

    emb_a = np.zeros((B, OUTER), np.float32)
    emb_b = np.zeros((B, OUTER), np.float32)
    for c in range(NCORES):
        e = res.results[c]["emb"].transpose(1, 0, 2).reshape(OUTER,
                                                            2 * NSLOTS)
        for s in range(NSLOTS):
            i = groups[s][c]
            ea, eb = e[:, 2 * s], e[:, 2 * s + 1]  # A-queries, B-queries
            if swap[i]:
                emb_a[i], emb_b[i] = eb, ea
            else:
                emb_a[i], emb_b[i] = ea, eb
    return emb_a, emb_b


# revision 23
# speedup vs baseline: 1.1884x; 1.1884x over previous
"""Ragged cross-attention pooling kernel for Trainium2 (8 NeuronCores, SPMD).

Math (per pair, direction "A attends over B"):
    qa = (A @ Wq + bq) * scale          [la, INNER]
    kb =  B @ Wk                        [lb, INNER]   (bk dropped: softmax
                                                       is shift-invariant per query)
    s  = qa @ kb^T                      [la, lb]      (pad k-cols are exactly 0)
    p  = exp(s)                                       (pad cols: exp(0) = 1.0)
    den[q] = sum_k p[q, k] - n_pad                    (exact pad correction)
    g[q] = valid(q) / (la * den[q])
    w[k] = sum_q g[q] p[q, k]           <- collapses the mean over queries
    emb  = (w^T B) @ Wv + bv            <- collapses attn@V and the V projection

Distribution: 64 pairs -> 8 slots x 8 cores (one shared SPMD program, shapes
fixed per slot to the max over cores; pairs bin-packed by length so padding is
small).

Perf notes vs the first version:
  - A/B uploaded BOTH pre-transposed (DIM-major, fp8e4m3, DIM zero-padded to
    768) for the Q/K path AND natural-layout bf16 for the value path; no
    on-device transposes at all.
  - Projections and QK^T run as fp8 DoubleRow matmuls (2 contraction rows per
    partition, 0.5 cyc/row).  INNER=256 = 2x128 maps exactly onto the
    DoubleRow pair dim for the scores.
  - Value path (w^T B, Wv^T u) in bf16 (1 cyc/row, no small-N penalty).
  - exp() is one activation per q-tile over a [128, plk] PSUM span with a
    single accumulator read for den.
"""

import os
import sys

sys.path.insert(0, "/opt/trn_rl_repo")

import numpy as np

B, LA, LB, DIM, INNER, OUTER = 64, 1024, 1024, 640, 256, 1024
NCORES, NSLOTS, P = 8, 8, 128
SCALE = 1.0 / np.sqrt(INNER)
DT = DIM // P       # 5 d-chunks of 128
DJ = 3              # DoubleRow d-pair chunks (768 = 3 * 256)
DPAD = DJ * 2 * P   # 768

LAST_EXEC_TIME_NS = None


def _chunks(total, step=512):
    out, off = [], 0
    while off < total:
        c = min(step, total - off)
        out.append((off, c))
        off += c
    return out


def _plan(la_all, lb_all):
    """Assign pairs to (slot, core); returns swap flags, groups, slot tile shapes."""
    la = np.asarray(la_all, np.int64)
    lb = np.asarray(lb_all, np.int64)
    swap = lb > la
    qa = np.where(swap, lb, la)  # kernel A-side length (>= B-side)
    qb = np.where(swap, la, lb)
    at = -(-qa // P)
    bt = -(-qb // P)
    order = np.argsort(-(at * 1024 + bt), kind="stable")
    groups = [list(order[s * NCORES:(s + 1) * NCORES]) for s in range(NSLOTS)]
    C1, C2 = 2000.0, 200.0

    def gcost(g):
        ma = max(at[i] for i in g)
        mb = max(bt[i] for i in g)
        return C1 * (ma + mb) + C2 * ma * mb

    rng = np.random.default_rng(0)
    cost = [gcost(g) for g in groups]
    s1s = rng.integers(0, NSLOTS, 30000)
    s2s = rng.integers(0, NSLOTS, 30000)
    i1s = rng.integers(0, NCORES, 30000)
    i2s = rng.integers(0, NCORES, 30000)
    for s1, s2, i1, i2 in zip(s1s, s2s, i1s, i2s):
        if s1 == s2:
            continue
        g1 = groups[s1][:]
        g2 = groups[s2][:]
        g1[i1], g2[i2] = groups[s2][i2], groups[s1][i1]
        n1, n2 = gcost(g1), gcost(g2)
        if n1 + n2 < cost[s1] + cost[s2] - 1e-9:
            groups[s1], groups[s2] = g1, g2
            cost[s1], cost[s2] = n1, n2
    slot_at = [int(max(at[i] for i in g)) for g in groups]
    slot_bt = [int(max(bt[i] for i in g)) for g in groups]
    return swap, qa, qb, groups, slot_at, slot_bt


def _build_program(slot_at, slot_bt):
    import concourse.bass as bass  # noqa: F401
    import concourse.mybir as mybir
    import concourse.tile as tile
    from concourse import bacc

    F32 = mybir.dt.float32
    F32R = mybir.dt.float32r
    BF16 = mybir.dt.bfloat16
    FP8 = mybir.dt.float8e4
    Exp = mybir.ActivationFunctionType.Exp
    Ident = mybir.ActivationFunctionType.Identity
    DR = mybir.MatmulPerfMode.DoubleRow
    Alu = mybir.AluOpType

    tot_at = sum(slot_at)
    tot_bt = sum(slot_bt)
    cum_at = np.concatenate([[0], np.cumsum(slot_at)]).astype(int)
    cum_bt = np.concatenate([[0], np.cumsum(slot_bt)]).astype(int)

    nc = bacc.Bacc("TRN2", target_bir_lowering=False, debug=False,
                   num_devices=NCORES)

    # transposed fp8 inputs: [p, j, i, tok] = X[tok, j*256 + i*128 + p]
    at8_d = nc.dram_tensor("at8", [P, DJ, 2, tot_at * P], FP8,
                           kind="ExternalInput")
    bt8_d = nc.dram_tensor("bt8", [P, DJ, 2, tot_bt * P], FP8,
                           kind="ExternalInput")
    # natural bf16 inputs: [p, T, d] = X[T*128 + p, d]
    an_d = nc.dram_tensor("an16", [P, tot_at, DIM], BF16, kind="ExternalInput")
    bn_d = nc.dram_tensor("bn16", [P, tot_bt, DIM], BF16, kind="ExternalInput")
    gs_a_d = nc.dram_tensor("gs_a", [P, tot_at], F32, kind="ExternalInput")
    gs_b_d = nc.dram_tensor("gs_b", [P, tot_bt], F32, kind="ExternalInput")
    npa_d = nc.dram_tensor("npa", [P, NSLOTS], F32, kind="ExternalInput")
    npb_d = nc.dram_tensor("npb", [P, NSLOTS], F32, kind="ExternalInput")
    wq_d = nc.dram_tensor("wq8", [P, DJ, 2, INNER], FP8, kind="ExternalInput")
    wk_d = nc.dram_tensor("wk8", [P, DJ, 2, INNER], FP8, kind="ExternalInput")
    wv_d = nc.dram_tensor("wv16", [P, DT, OUTER], BF16, kind="ExternalInput")
    bv_d = nc.dram_tensor("bv", [P, OUTER // P], F32, kind="ExternalInput")
    idr_d = nc.dram_tensor("idr", [P, P], F32R, kind="ExternalInput")
    idb_d = nc.dram_tensor("idb", [P, P], BF16, kind="ExternalInput")
    emb_d = nc.dram_tensor("emb", [P, OUTER // P, 2 * NSLOTS], F32,
                           kind="ExternalOutput")

    with tile.TileContext(nc) as tc:
        with (
            tc.tile_pool(name="const", bufs=1) as cpool,
            tc.tile_pool(name="ain", bufs=3) as apool,
            tc.tile_pool(name="proj", bufs=2) as ppool,
            tc.tile_pool(name="pexp", bufs=12) as epool,
            tc.tile_pool(name="small", bufs=7) as spool,
            tc.tile_pool(name="late", bufs=2) as lpool,
            tc.tile_pool(name="psA", bufs=3, space="PSUM") as psA,
            tc.tile_pool(name="psW", bufs=1, space="PSUM") as psW,
        ):
            # ---- constants (DMA-ordered: slot-0 critical path first) ----
            wq_sb = cpool.tile([P, DJ, 2, INNER], FP8, tag="wq")
            wk_sb = cpool.tile([P, DJ, 2, INNER], FP8, tag="wk")
            bv_sb = cpool.tile([P, OUTER // P], F32, tag="bv")
            idr_sb = cpool.tile([P, P], F32R, tag="idr")
            npa_sb = cpool.tile([P, NSLOTS], F32, tag="npa")
            npb_sb = cpool.tile([P, NSLOTS], F32, tag="npb")
            gs_a_sb = cpool.tile([P, tot_at], F32, tag="gsa")
            gs_b_sb = cpool.tile([P, tot_bt], F32, tag="gsb")
            wv_sb = cpool.tile([P, DT, OUTER], BF16, tag="wv")
            urows_sb = cpool.tile([2 * NSLOTS, DIM], F32R, tag="urows")
            for sb, d in ((wq_sb, wq_d), (wk_sb, wk_d)):
                nc.sync.dma_start(sb[:], d[:])

            inbufs = {}

            def load_slot(s, qk_only=False, nat_only=False):
                at_s, bt_s = int(slot_at[s]), int(slot_bt[s])
                if not nat_only:
                    a8 = apool.tile([P, DJ, 2, at_s * P], FP8, tag="a8")
                    b8 = apool.tile([P, DJ, 2, bt_s * P], FP8, tag="b8")
                    nc.sync.dma_start(
                        a8[:],
                        at8_d[:, :, :, cum_at[s] * P:(cum_at[s] + at_s) * P])
                    nc.sync.dma_start(
                        b8[:],
                        bt8_d[:, :, :, cum_bt[s] * P:(cum_bt[s] + bt_s) * P])
                    inbufs[s] = (a8, b8, None, None)
                if not qk_only:
                    an = apool.tile([P, at_s, DIM], BF16, tag="an")
                    bn = apool.tile([P, bt_s, DIM], BF16, tag="bn")
                    nc.sync.dma_start(an[:],
                                      an_d[:, cum_at[s]:cum_at[s] + at_s, :])
                    nc.sync.dma_start(bn[:],
                                      bn_d[:, cum_bt[s]:cum_bt[s] + bt_s, :])
                    a8, b8, _, _ = inbufs[s]
                    inbufs[s] = (a8, b8, an, bn)

            projbufs = {}

            def proj_gen(s):
                """fp8 DoubleRow projections: qT/kT [p, m, tok]."""
                at_s, bt_s = int(slot_at[s]), int(slot_bt[s])
                pla, plb = at_s * P, bt_s * P
                a8, b8, an, bn = inbufs.pop(s)
                qaT = ppool.tile([P, 2, pla], FP8, tag="qaT")
                kaT = ppool.tile([P, 2, pla], FP8, tag="kaT")
                qbT = ppool.tile([P, 2, plb], FP8, tag="qbT")
                kbT = ppool.tile([P, 2, plb], FP8, tag="kbT")
                projbufs[s] = (qaT, kaT, qbT, kbT, an, bn)
                for src, pl, dst, w_sb in (
                        (a8, pla, qaT, wq_sb),
                        (a8, pla, kaT, wk_sb),
                        (b8, plb, qbT, wq_sb),
                        (b8, plb, kbT, wk_sb)):
                    for m in range(2):
                        pp = psA.tile([P, 1024], F32, tag="mm")
                        for j in range(DJ):
                            for co, cl in _chunks(pl):
                                nc.tensor.matmul(
                                    pp[:, co:co + cl],
                                    w_sb[:, j, :, m * P:(m + 1) * P],
                                    src[:, j, :, co:co + cl],
                                    start=(j == 0), stop=(j == DJ - 1),
                                    perf_mode=DR)
                        # plain fp8 cast: the q bias rides the ones-row of
                        # A/Wq, the softmax scale rides the exp activation,
                        # and k needs no bias (softmax shift-invariance)
                        nc.vector.tensor_copy(dst[:, m, :], pp[:, :pl])
                        yield

            def tail_gen(s, dr, wr, plk, nk, knat):
                """Deferred per-direction epilogue: transpose w, compute u.

                First step (run eagerly at direction end): wrow copy, freeing
                the wr psum slot.  Later steps are drained one per q-tile
                of the following direction so the PE never idles on the
                wrow/wcol dependency chain.
                """
                wrow = lpool.tile([1, 1024], F32R, tag="wrow")
                if dr == 0:
                    nc.scalar.copy(wrow[0:1, :plk], wr[0:1, :plk])
                else:
                    nc.vector.tensor_copy(wrow[0:1, :plk], wr[0:1, :plk])
                yield
                wt = psA.tile([P, 1024], F32, tag="mm")
                for kt in range(nk):
                    nc.tensor.matmul(
                        wt[:, 2 * kt:2 * kt + 2],
                        wrow[0:1, kt * P:(kt + 1) * P],
                        idr_sb[0:1, 0:2], start=True, stop=True)
                wcol = spool.tile([P, 8], BF16, tag="wcol")
                nc.vector.tensor_copy(
                    wcol[:, :nk],
                    wt[:, :2 * nk].rearrange(
                        "p (k two) -> p k two", two=2)[:, :, 0])
                yield
                # u row = w^T @ Knat   (bf16); ur reuses the wr psum slot
                ur = psW.tile([1, 1024], F32, tag="wr")
                for co, cl in _chunks(DIM):
                    for kt in range(nk):
                        nc.tensor.matmul(
                            ur[0:1, co:co + cl],
                            wcol[:, kt:kt + 1],
                            knat[:, kt, co:co + cl],
                            start=(kt == 0), stop=(kt == nk - 1))
                ursb = lpool.tile([1, DIM], F32R, tag="ursb")
                if dr == 0:
                    nc.vector.tensor_copy(ursb[:], ur[0:1, :DIM])
                else:
                    nc.scalar.copy(ursb[:], ur[0:1, :DIM])
                nc.sync.dma_start(urows_sb[2 * s + dr:2 * s + dr + 1, :],
                                  ursb[:])

            # unified deferred-work queue: wacc pairs and direction tails are
            # issued ~2 q-tiles late (crossing direction/slot boundaries) so
            # the scalar->vector dependency chain never stalls the in-order
            # PE queue
            work = []

            def drain_work(slack):
                # Pops exhausted items freely; steps the head generator, but
                # keeps stepping whenever the backlog exceeds the hard bound
                # so ring-buffer reuse distances stay within the pool sizes.
                while len(work) > slack:
                    gen = work[0]
                    if next(gen, StopIteration) is StopIteration:
                        work.pop(0)
                        continue
                    if slack and len(work) <= 4:
                        break

            def attn_gen(s):
                at_s, bt_s = int(slot_at[s]), int(slot_bt[s])
                qaT, kaT, qbT, kbT, an, bn = projbufs.pop(s)
                for dr in range(2):
                    if dr == 0:  # A queries over B keys
                        QT, KT, nq, nk = qaT, kbT, at_s, bt_s
                        g_sb, g_off = gs_a_sb, cum_at[s]
                        np_sb = npb_sb
                        knat = bn
                    else:
                        QT, KT, nq, nk = qbT, kaT, bt_s, at_s
                        g_sb, g_off = gs_b_sb, cum_bt[s]
                        np_sb = npa_sb
                        knat = an
                    plk = nk * P
                    kch = _chunks(plk)
                    wr = psW.tile([1, 1024], F32, tag="wr")
                    den2 = None

                    def wacc_gen(q0, qn, gcol2, ptiles, wr=wr, kch=kch,
                                 nq=nq):
                        for qp in range(q0, qn + 1):
                            pt = ptiles[qp]
                            for co, cl in kch:
                                nc.tensor.matmul(
                                    wr[0:1, co:co + cl],
                                    gcol2[:, qp - q0:qp - q0 + 1],
                                    pt[:, co:co + cl],
                                    start=(qp == 0), stop=(qp == nq - 1))
                        return
                        yield

                    p_tiles = {}
                    for qt in range(nq):
                        sc = psA.tile([P, 1024], F32, tag="mm")
                        for co, cl in kch:
                            nc.tensor.matmul(
                                sc[:, co:co + cl],
                                QT[:, :, qt * P:(qt + 1) * P],
                                KT[:, :, co:co + cl],
                                start=True, stop=True, perf_mode=DR)
                        if qt % 2 == 0:
                            den2 = spool.tile([P, 2], F32, tag="den")
                        p_sb = epool.tile([P, 1024], BF16, tag="p")
                        p_tiles[qt] = p_sb
                        # p = exp(s / sqrt(INNER)); the softmax scale rides
                        # the activation, the q bias rides the ones-row of A
                        nc.scalar.activation(
                            p_sb[:, :plk], sc[:, :plk], Exp, scale=SCALE,
                            accum_out=den2[:, qt % 2:qt % 2 + 1])
                        if qt % 2 == 1 or qt == nq - 1:
                            q0 = qt - (qt % 2)
                            npair = qt - q0 + 1
                            dpair = den2[:, :npair]
                            # den -= pad count (pad cols are exactly exp(0)=1)
                            nc.vector.tensor_scalar_sub(
                                dpair, dpair, np_sb[:, s:s + 1])
                            rec2 = spool.tile([P, 2], F32, tag="rec")
                            nc.vector.reciprocal(rec2[:, :npair], dpair)
                            gcol2 = spool.tile([P, 2], BF16, tag="gc")
                            nc.vector.tensor_tensor(
                                gcol2[:, :npair], rec2[:, :npair],
                                g_sb[:, g_off + q0:g_off + q0 + npair],
                                Alu.mult)
                            work.append(wacc_gen(q0, qt, gcol2, p_tiles))
                            p_tiles = {}
                        drain_work(3)
                        yield
                    work.append(tail_gen(s, dr, wr, plk, nk, knat))
                    yield

            # software pipeline: slot attention interleaved with the next
            # slot's projections; input DMA prefetched ~two slots ahead;
            # slots processed smallest-first so the pipeline warm-up bubble
            # is as short as possible
            sorder = sorted(range(NSLOTS),
                            key=lambda s: slot_at[s] + slot_bt[s])
            load_slot(sorder[0], qk_only=True)
            for sb, d in ((gs_a_sb, gs_a_d), (gs_b_sb, gs_b_d),
                          (npa_sb, npa_d), (npb_sb, npb_d),
                          (idr_sb, idr_d)):
                nc.sync.dma_start(sb[:], d[:])
            load_slot(sorder[0], nat_only=True)
            load_slot(sorder[1])
            nc.sync.dma_start(bv_sb[:], bv_d[:])
            nc.sync.dma_start(wv_sb[:], wv_d[:])
            for _ in proj_gen(sorder[0]):
                pass
            for i, s in enumerate(sorder):
                if i + 2 < NSLOTS:
                    load_slot(sorder[i + 2])
                ag = attn_gen(s)
                pg = proj_gen(sorder[i + 1]) if i + 1 < NSLOTS else None
                for _ in ag:
                    if pg is not None:
                        if next(pg, StopIteration) is StopIteration:
                            pg = None
                if pg is not None:
                    for _ in pg:
                        pass
            drain_work(0)

            # ---- final: E = Wv^T U + bv ----
            u_sb = cpool.tile([P, DT, 2 * NSLOTS], BF16, tag="usb")
            for dt in range(DT):
                ut = psA.tile([P, 1024], F32, tag="mm")
                nc.tensor.matmul(
                    ut[:, :2 * NSLOTS],
                    urows_sb[:, dt * P:(dt + 1) * P],
                    idr_sb[0:2 * NSLOTS, 0:2 * NSLOTS],
                    start=True, stop=True)
                nc.vector.tensor_copy(u_sb[:, dt, :], ut[:, :2 * NSLOTS])
            e_sb = cpool.tile([P, OUTER // P, 2 * NSLOTS], F32, tag="esb")
            for oc in range(OUTER // P):
                ep = psA.tile([P, 1024], F32, tag="mm")
                for dt in range(DT):
                    nc.tensor.matmul(
                        ep[:, :2 * NSLOTS],
                        wv_sb[:, dt, oc * P:(oc + 1) * P],
                        u_sb[:, dt, :],
                        start=(dt == 0), stop=(dt == DT - 1))
                nc.vector.tensor_scalar_add(e_sb[:, oc, :], ep[:, :2 * NSLOTS],
                                            bv_sb[:, oc, None])
            nc.sync.dma_start(emb_d[:], e_sb[:])

    nc.compile()
    return nc


def _install_profhook():
    import contextlib
    import ctypes
    import types

    import antenv

    if not hasattr(antenv, "axon_hooks"):
        mod = types.ModuleType("antenv.axon_hooks")
        mod._hook = None

        def _set(h):
            mod._hook = h

        def _get():
            return mod._hook

        mod.set_axon_ntff_profile_hook = _set
        mod.get_axon_ntff_profile_hook = _get
        sys.modules["antenv.axon_hooks"] = mod
        antenv.axon_hooks = mod
    from antenv.axon_hooks import set_axon_ntff_profile_hook
    so_path = "/opt/axon/libaxon_pjrt.so"
    if not os.path.exists(so_path):
        return False
    lib = ctypes.CDLL(so_path)
    if not hasattr(lib, "axon_start_nrt_profile"):
        return False
    lib.axon_start_nrt_profile.argtypes = [ctypes.POINTER(ctypes.c_int64),
                                           ctypes.c_size_t]
    lib.axon_start_nrt_profile.restype = ctypes.c_int64
    lib.axon_stop_nrt_profile.argtypes = [ctypes.c_char_p]
    lib.axon_stop_nrt_profile.restype = ctypes.c_int64

    @contextlib.contextmanager
    def _hook(output_dir, device_ids):
        import jax

        jax.devices()
        if device_ids:
            ids = (ctypes.c_int64 * len(device_ids))(*device_ids)
            rc = lib.axon_start_nrt_profile(ids, len(device_ids))
        else:
            rc = lib.axon_start_nrt_profile(None, 0)
        if rc != 0:
            raise RuntimeError(f"axon_start_nrt_profile rc={rc}")
        try:
            yield
        finally:
            n = lib.axon_stop_nrt_profile(str(output_dir).encode())
            print(f"profile: {n} file(s) written to {output_dir}",
                  file=sys.stderr)

    set_axon_ntff_profile_hook(_hook)
    return True


def kernel(a_pad, b_pad, len_a, len_b, Wq, bq, Wk, bk, Wv, bv):
    global LAST_EXEC_TIME_NS
    import ml_dtypes
    FP8 = ml_dtypes.float8_e4m3fn
    BF16 = ml_dtypes.bfloat16

    a_pad = np.ascontiguousarray(np.asarray(a_pad, np.float32))
    b_pad = np.ascontiguousarray(np.asarray(b_pad, np.float32))
    len_a = np.asarray(len_a, np.int32)
    len_b = np.asarray(len_b, np.int32)
    Wq = np.asarray(Wq, np.float32)
    Wk = np.asarray(Wk, np.float32)
    Wv = np.asarray(Wv, np.float32)
    bq = np.asarray(bq, np.float32)
    bv = np.asarray(bv, np.float32)

    swap, qa_len, qb_len, groups, slot_at, slot_bt = _plan(len_a, len_b)
    tot_at, tot_bt = sum(slot_at), sum(slot_bt)
    cum_at = np.concatenate([[0], np.cumsum(slot_at)]).astype(int)
    cum_bt = np.concatenate([[0], np.cumsum(slot_bt)]).astype(int)

    # ---- shared (per-core-identical) inputs ----
    def pack_w8(W, brow=None):
        # [640, INNER] -> [128, 3, 2, INNER] with d = j*256 + i*128 + p;
        # row DIM carries the bias (the data carries 1.0 there)
        Wp = np.zeros((DPAD, W.shape[1]), np.float32)
        Wp[:DIM] = W
        if brow is not None:
            Wp[DIM] = brow
        return np.ascontiguousarray(
            Wp.reshape(DJ, 2, P, W.shape[1]).transpose(2, 0, 1, 3)
        ).astype(FP8)

    wq8 = pack_w8(Wq, bq)
    wk8 = pack_w8(Wk)
    wv16 = np.ascontiguousarray(
        Wv.reshape(DT, P, OUTER).transpose(1, 0, 2)).astype(BF16)
    bv_h = bv.reshape(OUTER // P, P).T.copy()
    idr_h = np.eye(P, dtype=np.float32)
    idb_h = np.eye(P, dtype=np.float32).astype(BF16)

    # ---- per-core inputs ----
    in_maps = []
    for c in range(NCORES):
        abuf = np.zeros((tot_at * P, DPAD), np.float32)
        bbuf = np.zeros((tot_bt * P, DPAD), np.float32)
        abuf[:, DIM] = 1.0
        bbuf[:, DIM] = 1.0
        gs_a = np.zeros((P, tot_at), np.float32)
        gs_b = np.zeros((P, tot_bt), np.float32)
        npa = np.zeros((P, NSLOTS), np.float32)
        npb = np.zeros((P, NSLOTS), np.float32)
        for s in range(NSLOTS):
            i = groups[s][c]
            la_i, lb_i = int(qa_len[i]), int(qb_len[i])
            A = b_pad[i] if swap[i] else a_pad[i]
            Bm = a_pad[i] if swap[i] else b_pad[i]
            abuf[cum_at[s] * P:cum_at[s] * P + la_i, :DIM] = A[:la_i]
            bbuf[cum_bt[s] * P:cum_bt[s] * P + lb_i, :DIM] = Bm[:lb_i]
            ga = np.zeros(slot_at[s] * P, np.float32)
            ga[:la_i] = 1.0 / la_i
            gs_a[:, cum_at[s]:cum_at[s] + slot_at[s]] = \
                ga.reshape(slot_at[s], P).T
            gb = np.zeros(slot_bt[s] * P, np.float32)
            gb[:lb_i] = 1.0 / lb_i
            gs_b[:, cum_bt[s]:cum_bt[s] + slot_bt[s]] = \
                gb.reshape(slot_bt[s], P).T
            npa[:, s] = slot_at[s] * P - la_i
            npb[:, s] = slot_bt[s] * P - lb_i
        # transposed fp8: [tok, 768] -> [128, 3, 2, tok]
        at8 = np.ascontiguousarray(
            abuf.reshape(tot_at * P, DJ, 2, P).transpose(3, 1, 2, 0)
        ).astype(FP8)
        bt8 = np.ascontiguousarray(
            bbuf.reshape(tot_bt * P, DJ, 2, P).transpose(3, 1, 2, 0)
        ).astype(FP8)
        # natural bf16: [tok, 640] -> [128, T, 640]
        an16 = np.ascontiguousarray(
            abuf[:, :DIM].reshape(tot_at, P, DIM).transpose(1, 0, 2)
        ).astype(BF16)
        bn16 = np.ascontiguousarray(
            bbuf[:, :DIM].reshape(tot_bt, P, DIM).transpose(1, 0, 2)
        ).astype(BF16)
        in_maps.append({
            "at8": at8, "bt8": bt8, "an16": an16, "bn16": bn16,
            "gs_a": gs_a, "gs_b": gs_b, "npa": npa, "npb": npb,
            "wq8": wq8, "wk8": wk8, "wv16": wv16,
            "bv": bv_h, "idr": idr_h, "idb": idb_h,
        })

    nc = _build_program(slot_at, slot_bt)

    from concourse.bass_utils import run_bass_kernel_spmd

    trace = os.environ.get("BASS_KERNEL_TRACE", "0") == "1"
    if trace:
        _install_profhook()
    res = run_bass_kernel_spmd(nc, in_maps, list(range(NCORES)), trace=trace)
    LAST_EXEC_TIME_NS = res.exec_time_ns

    emb_a = np.zeros((B, OUTER), np.float32)
    emb_b = np.zeros((B, OUTER), np.float32)
    for c in range(NCORES):
        e = res.results[c]["emb"].transpose(1, 0, 2).reshape(OUTER,
                                                            2 * NSLOTS)
        for s in range(NSLOTS):
            i = groups[s][c]
            ea, eb = e[:, 2 * s], e[:, 2 * s + 1]  # A-queries, B-queries
            if swap[i]:
                emb_a[i], emb_b[i] = eb, ea
            else:
                emb_a[i], emb_b[i] = ea, eb
    return emb_a, emb_b
